# revision 21
# baseline (speedup 1.0000x reference)
"""CrossAttention kernel for 8 Trainium2 NeuronCores.

Data-parallel over batch: B=16 batches -> 2 per core. Each core computes the
full cross-attention for its 2 batches; outputs are concatenated on host.

Per-core dataflow (all matmuls fp16 in / fp32 psum, full-rate on the PE):
  x^T    host-pretiled [b, t, 128, 4, TQ]
  Q^T    = Wq^T @ x^T            (Wq pre-scaled by 1/8 on host)
  ctx^T  host-pretransposed -> K^T = Wk^T @ ctx^T,  V = ctx @ Wv
  S^T    = (K_h^T).T @ Q_h^T     [77, nq]  per head
  expS   = exp(S^T)              (no max-subtract; |S| <~ 6 so fp32 exp safe)
  den    = ones^T @ expS         [64, nq] replicated rows, pair-packed psum
  O^T    = V_h.T @ expS          [64, nq], head pairs packed into [128, nq]
  norm   : O^T * recip(den)      (DVE)
  out    = (O^T).T @ Wo          [nq, 512] fp16 tile-major; host adds bias
           and casts to fp32.

All DRAM tensors are host-side pre-tiled so each DMA is contiguous >=2KB per
partition (few descriptors). PE program starts with Q-proj of tile 0 (needs
only wq+xT0) to cover the wk/wv/wo load latency.
"""

import os
import sys

for _p in ("/opt/trn_rl_repo",):
    if _p not in sys.path:
        sys.path.insert(0, _p)

import numpy as np

import concourse.bass as bass
import concourse.bacc as bacc
import concourse.mybir as mybir
import concourse.tile as tile
from concourse.bass_utils import run_bass_kernel_spmd

# Problem constants (hardcoded per contract)
B, NQ, NK = 16, 4096, 77
DQ, DC = 512, 768
H, DH = 8, 64
INNER = H * DH  # 512
SCALE = DH ** -0.5  # 1/8
NCORES = 8
BLOC = B // NCORES  # 2 batches per core

F32 = mybir.dt.float32
F16 = mybir.dt.float16

TQ = 512          # nq tile (free dim of most matmuls)
NTILES = NQ // TQ  # 8 per batch
KQ = DQ // 128    # 4 contraction chunks for Wq
KC = DC // 128    # 6 contraction chunks for Wk/Wv
CI = INNER // 128  # 4 inner chunks


def _build_nc():
    nc = bacc.Bacc("TRN2", target_bir_lowering=False, debug=False)

    # host-pretiled inputs: contiguous per-partition rows
    xT_l = nc.dram_tensor(
        "xT_l", [BLOC, NTILES, 128, KQ, TQ], F16, kind="ExternalInput")
    ctxT_l = nc.dram_tensor(
        "ctxT_l", [128, KC, BLOC * NK], F16, kind="ExternalInput")
    wq = nc.dram_tensor("wq", [128, KQ, INNER], F16, kind="ExternalInput")
    wk = nc.dram_tensor("wk", [128, KC, INNER], F16, kind="ExternalInput")
    wv = nc.dram_tensor("wv", [128, KC, INNER], F16, kind="ExternalInput")
    wo = nc.dram_tensor("wo", [128, CI, DQ], F16, kind="ExternalInput")
    ones77 = nc.dram_tensor("ones77", [NK, 64], F16, kind="ExternalInput")
    out_l = nc.dram_tensor(
        "out_l", [BLOC, NTILES, 128, 4, DQ], F16, kind="ExternalOutput")

    with tile.TileContext(nc) as tc:
        with (
            tc.tile_pool(name="consts", bufs=1) as consts,
            tc.tile_pool(name="kv", bufs=1) as kv,
            tc.tile_pool(name="xp", bufs=3) as xp,
            tc.tile_pool(name="ep", bufs=12) as ep,
            tc.tile_pool(name="rp", bufs=8) as rp,
            tc.tile_pool(name="op", bufs=2) as op,
            tc.tile_pool(name="fp", bufs=3) as fp,
            tc.tile_pool(name="ps", bufs=8, space="PSUM") as ps,
        ):
            # ---- loads, ordered by first use; wq/xT0 split by k-chunk so
            # the first Q matmul starts as soon as chunk 0 lands.  The sync
            # queue dispatches DMAs ~3us later than scalar/vector/gpsimd at
            # startup, so the critical first chunks go on scalar+vector.
            # each dma_start costs ~1.7us of trigger latency on its queue, so
            # keep the startup loads UNSPLIT and spread by first-use across
            # the scalar/gpsimd queues (sync dispatches ~3us later).
            wq_sb = consts.tile([128, KQ, INNER], F16)
            nc.scalar.dma_start(wq_sb[:, 0:2, :], wq[:, 0:2])
            xT0_sb = xp.tile([128, KQ, TQ], F16, tag="xT")
            nc.sync.dma_start(xT0_sb[:], xT_l[0, 0])
            nc.scalar.dma_start(wq_sb[:, 2:4, :], wq[:, 2:4])
            ctxT_sb = kv.tile([128, KC, BLOC * NK], F16)
            nc.scalar.dma_start(ctxT_sb[:], ctxT_l.ap())
            wk_sb = consts.tile([128, KC, INNER], F16)
            nc.gpsimd.dma_start(wk_sb[:], wk.ap())
            ones77_sb = consts.tile([NK, 64], F16)
            nc.gpsimd.dma_start(ones77_sb[:], ones77[:])
            wv_sb = consts.tile([128, KC, INNER], F16)
            nc.gpsimd.dma_start(wv_sb[:], wv.ap())
            xT1_sb = xp.tile([128, KQ, TQ], F16, tag="xT")
            nc.sync.dma_start(xT1_sb[:], xT_l[0, 1])
            wo_sb = consts.tile([128, CI, DQ], F16)
            nc.gpsimd.dma_start(wo_sb[:], wo.ap())

            # ---- Q-proj of tile 0 first: only needs wq + xT0.  k-outer so
            # the first matmuls start when just the k=0 chunks have landed.
            qt0_sb = xp.tile([128, CI, TQ], F16, tag="qt")
            q0_pss = [ps.tile([128, TQ], F32, tag="ps", name=f"q0_ps{c}")
                      for c in range(CI)]
            for k in range(KQ):
                for c in range(CI):
                    nc.tensor.matmul(
                        q0_pss[c][:], wq_sb[:, k, c * 128:(c + 1) * 128],
                        xT0_sb[:, k, :], start=(k == 0), stop=(k == KQ - 1))
            for c in range(CI):
                nc.scalar.activation(
                    qt0_sb[:, c, :], q0_pss[c][:],
                    mybir.ActivationFunctionType.Copy)

            # ---- context projections (K^T both batches, V per batch).
            # k-outer so the first matmuls need only wk chunk 0.
            kt_sb = kv.tile([128, CI, BLOC * NK], F16)
            kt_pss = [ps.tile([128, BLOC * NK], F32, tag="ps",
                              name=f"kt_ps{c}") for c in range(CI)]
            for k in range(KC):
                for c in range(CI):
                    nc.tensor.matmul(
                        kt_pss[c][:], wk_sb[:, k, c * 128:(c + 1) * 128],
                        ctxT_sb[:, k, :], start=(k == 0), stop=(k == KC - 1))
            for c in range(CI):
                nc.scalar.activation(
                    kt_sb[:, c, :], kt_pss[c][:],
                    mybir.ActivationFunctionType.Copy)

            v_sb = kv.tile([NK, BLOC, INNER], F16)
            for b in range(BLOC):
                v_ps = ps.tile([NK, INNER], F32, tag="ps")
                for k in range(KC):
                    nc.tensor.matmul(
                        v_ps[:], ctxT_sb[:, k, b * NK:(b + 1) * NK],
                        wv_sb[:, k, :], start=(k == 0), stop=(k == KC - 1))
                nc.scalar.activation(
                    v_sb[:, b, :], v_ps[:],
                    mybir.ActivationFunctionType.Copy)

            # ---- main loop over (batch, nq tile), software-pipelined:
            # per iteration: S/exp(t), O/den/norm(t), Q-proj(t+1), out(t).
            # Q(t+1) between attention and out keeps the psum ring reusing
            # slots that were evicted mid-tile, and gives the PE work while
            # pair-3's normalization completes.
            NT = BLOC * NTILES

            def q_chunk(ti_next, qt_tag_sb, xT_sb, c):
                q_ps = ps.tile([128, TQ], F32, tag="ps",
                               name=f"q_ps_{ti_next}_{c}")
                for k in range(KQ):
                    nc.tensor.matmul(
                        q_ps[:], wq_sb[:, k, c * 128:(c + 1) * 128],
                        xT_sb[:, k, :], start=(k == 0), stop=(k == KQ - 1))
                nc.scalar.activation(
                    qt_tag_sb[:, c, :], q_ps[:],
                    mybir.ActivationFunctionType.Copy)

            qt_cur = qt0_sb
            for ti in range(NT):
                b, t = divmod(ti, NTILES)
                qt_sb = qt_cur

                # prefetch x^T of tile t+1 early
                if ti + 1 < NT:
                    bn, tn = divmod(ti + 1, NTILES)
                    if ti + 1 == 1:
                        xT_next = xT1_sb
                    else:
                        xT_next = xp.tile([128, KQ, TQ], F16, tag="xT",
                                          name=f"xT_{ti + 1}")
                        if (ti + 1) % 2 == 0:
                            nc.sync.dma_start(xT_next[:], xT_l[bn, tn])
                        else:
                            nc.gpsimd.dma_start(xT_next[:], xT_l[bn, tn])
                    qt_cur = xp.tile([128, CI, TQ], F16, tag="qt",
                                     name=f"qt_{ti + 1}")

                # attention: S^T, exp
                e_sbs = []
                for h in range(H):
                    c, r = h // 2, (h % 2) * 64
                    s_ps = ps.tile([NK, TQ], F32, tag="ps",
                                   name=f"s_ps_{ti}_{h}")
                    nc.tensor.matmul(
                        s_ps[:],
                        kt_sb[r:r + DH, c, b * NK:(b + 1) * NK],
                        qt_sb[r:r + DH, c, :])
                    e_sb = ep.tile([NK, TQ], F16, tag="expS",
                                   name=f"e_sb_{ti}_{h}")
                    nc.scalar.activation(
                        e_sb[:], s_ps[:], mybir.ActivationFunctionType.Exp)
                    e_sbs.append(e_sb)

                # O^T head-pairs packed [128, TQ], normalized by 1/den.
                # Q-proj chunks of tile t+1 are interleaved between pairs so
                # the PE never waits on the serial exp chain (pairs 2-3
                # would otherwise race exp5/exp7).
                ot_sb = op.tile([128, CI, TQ], F16, tag="ot",
                                name=f"ot_{ti}")
                for g in range(H // 2):
                    o2_ps = ps.tile([128, TQ], F32, tag="ps",
                                    name=f"o2_ps_{ti}_{g}")
                    d_ps = ps.tile([128, TQ], F32, tag="ps",
                                   name=f"d_ps_{ti}_{g}")
                    for half in range(2):
                        h = 2 * g + half
                        nc.tensor.matmul(
                            o2_ps[half * 64:(half + 1) * 64, :],
                            v_sb[:, b, h * DH:(h + 1) * DH],
                            e_sbs[h][:])
                        nc.tensor.matmul(
                            d_ps[half * 64:(half + 1) * 64, :],
                            ones77_sb[:], e_sbs[h][:],
                            tile_position=(0, half * 64))
                    rdbc = rp.tile([128, TQ], F32, tag="rdbc",
                                   name=f"rdbc_{ti}_{g}")
                    nc.vector.reciprocal_approx_fast(rdbc[:], d_ps[:])
                    nc.vector.tensor_mul(ot_sb[:, g, :], o2_ps[:], rdbc[:])
                    if ti + 1 < NT:
                        # one Q chunk of tile t+1 after every pair: keeps the
                        # PE off both the exp-chain race and the psum-ring
                        # WAR (pair allocs land ~1us after their slot frees)
                        q_chunk(ti + 1, qt_cur, xT_next, g)

                # out = (O^T).T @ Wo   (bias added on host)
                # c-outer / j-inner so the c=3 wave starts well after
                # pair-3's normalization.
                f_sb = fp.tile([128, 4, DQ], F16, tag="fin",
                               name=f"f_sb_{ti}")
                f_pss = [ps.tile([128, DQ], F32, tag="ps",
                                 name=f"f_ps_{ti}_{j}")
                         for j in range(4)]
                for c in range(CI):
                    for j in range(4):
                        nc.tensor.matmul(
                            f_pss[j][:], ot_sb[:, c, j * 128:(j + 1) * 128],
                            wo_sb[:, c, :],
                            start=(c == 0), stop=(c == CI - 1))
                # final evictions on DVE (keeping ACT clear for the next
                # tile's exp chain); last tile splits ACT/DVE and streams
                # each chunk out as soon as it is evicted
                last = ti == NT - 1
                for j in range(4):
                    if last and j % 2 == 0:
                        nc.scalar.activation(
                            f_sb[:, j, :], f_pss[j][:],
                            mybir.ActivationFunctionType.Copy)
                    else:
                        nc.vector.tensor_copy(f_sb[:, j, :], f_pss[j][:])
                    if last:
                        nc.sync.dma_start(out_l[b, t, :, j], f_sb[:, j, :])
                if not last:
                    st_eng = [nc.sync, nc.gpsimd][ti % 2]
                    st_eng.dma_start(out_l[b, t], f_sb[:])

    nc.compile()
    return nc


_NC_CACHE = {}


def _get_nc():
    if "nc" not in _NC_CACHE:
        _NC_CACHE["nc"] = _build_nc()
    return _NC_CACHE["nc"]


def _make_in_maps(x, context, Wq, Wk, Wv, Wo):
    f = np.float32

    def wtile(w, kchunks):
        # [K, N] -> [128, kchunks, N] with partition p holding rows
        # {p, 128+p, ...}? No: row k*128+p -> [p, k, n]
        w = np.ascontiguousarray(w, dtype=f).astype(np.float16)
        return np.ascontiguousarray(
            w.reshape(kchunks, 128, -1).transpose(1, 0, 2))

    wq_t = np.ascontiguousarray(Wq, dtype=f) * np.float32(SCALE)
    shared = {
        "wq": wtile(wq_t, KQ),
        "wk": wtile(np.asarray(Wk, dtype=f), KC),
        "wv": wtile(np.asarray(Wv, dtype=f), KC),
        "wo": wtile(np.asarray(Wo, dtype=f), CI),
        "ones77": np.ones((NK, 64), dtype=np.float16),
    }
    in_maps = []
    for i in range(NCORES):
        m = dict(shared)
        xc = np.asarray(x[BLOC * i:BLOC * (i + 1)], dtype=f)
        # x [b, nq, dq] -> xT [b, t, p, c, n] = x[b, t*TQ+n, c*128+p]
        xt = xc.reshape(BLOC, NTILES, TQ, KQ, 128).transpose(0, 1, 4, 3, 2)
        m["xT_l"] = np.ascontiguousarray(xt).astype(np.float16)
        cc = np.asarray(context[BLOC * i:BLOC * (i + 1)], dtype=f)
        # ctx [b, nk, dc] -> ctxT [p, k, b*NK+n] = ctx[b, n, k*128+p]
        ct = cc.reshape(BLOC, NK, KC, 128).transpose(3, 2, 0, 1).reshape(
            128, KC, BLOC * NK)
        m["ctxT_l"] = np.ascontiguousarray(ct).astype(np.float16)
        in_maps.append(m)
    return in_maps


def run(x, context, Wq, Wk, Wv, Wo, bo, trace=False, **trace_kwargs):
    nc = _get_nc()
    in_maps = _make_in_maps(x, context, Wq, Wk, Wv, Wo)
    res = run_bass_kernel_spmd(
        nc, in_maps, list(range(NCORES)), trace=trace, **trace_kwargs)
    parts = []
    for i in range(NCORES):
        o = np.asarray(res.results[i]["out_l"])  # [BLOC, NTILES, 128, 4, DQ]
        o = o.transpose(0, 1, 3, 2, 4).reshape(BLOC, NQ, DQ)
        parts.append(o)
    out = np.concatenate(parts, axis=0).astype(np.float32)
    out += np.asarray(bo, dtype=np.float32)[None, None, :]
    return out, res


def kernel(x, context, Wq, Wk, Wv, Wo, bo):
    out, _ = run(x, context, Wq, Wk, Wv, Wo, bo, trace=False)
    return out


# revision 23
# speedup vs baseline: 1.0158x; 1.0158x over previous
"""CrossAttention kernel for 8 Trainium2 NeuronCores.

Data-parallel over batch: B=16 batches -> 2 per core. Each core computes the
full cross-attention for its 2 batches; outputs are concatenated on host.

Per-core dataflow (all matmuls fp16 in / fp32 psum, full-rate on the PE):
  x^T    host-pretiled [b, t, 128, 4, TQ]
  Q^T    = Wq^T @ x^T            (Wq pre-scaled by 1/8 on host)
  ctx^T  host-pretransposed -> K^T = Wk^T @ ctx^T,  V = ctx @ Wv
  S^T    = (K_h^T).T @ Q_h^T     [77, nq]  per head
  expS   = exp(S^T)              (no max-subtract; |S| <~ 6 so fp32 exp safe)
  den    = ones^T @ expS         [64, nq] replicated rows, pair-packed psum
  O^T    = V_h.T @ expS          [64, nq], head pairs packed into [128, nq]
  norm   : O^T * recip(den)      (DVE)
  out    = (O^T).T @ Wo          [nq, 512] fp16 tile-major; host adds bias
           and casts to fp32.

All DRAM tensors are host-side pre-tiled so each DMA is contiguous >=2KB per
partition (few descriptors). PE program starts with Q-proj of tile 0 (needs
only wq+xT0) to cover the wk/wv/wo load latency.
"""

import os
import sys

for _p in ("/opt/trn_rl_repo",):
    if _p not in sys.path:
        sys.path.insert(0, _p)

import numpy as np

import concourse.bass as bass
import concourse.bacc as bacc
import concourse.mybir as mybir
import concourse.tile as tile
from concourse.bass_utils import run_bass_kernel_spmd

# Problem constants (hardcoded per contract)
B, NQ, NK = 16, 4096, 77
DQ, DC = 512, 768
H, DH = 8, 64
INNER = H * DH  # 512
SCALE = DH ** -0.5  # 1/8
NCORES = 8
BLOC = B // NCORES  # 2 batches per core

F32 = mybir.dt.float32
F16 = mybir.dt.float16

TQ = 512          # nq tile (free dim of most matmuls)
NTILES = NQ // TQ  # 8 per batch
KQ = DQ // 128    # 4 contraction chunks for Wq
KC = DC // 128    # 6 contraction chunks for Wk/Wv
CI = INNER // 128  # 4 inner chunks


def _build_nc():
    nc = bacc.Bacc("TRN2", target_bir_lowering=False, debug=False)

    # host-pretiled inputs: contiguous per-partition rows
    xT_l = nc.dram_tensor(
        "xT_l", [BLOC, NTILES, 128, KQ, TQ], F16, kind="ExternalInput")
    ctxT_l = nc.dram_tensor(
        "ctxT_l", [128, KC, BLOC * NK], F16, kind="ExternalInput")
    wq = nc.dram_tensor("wq", [128, KQ, INNER], F16, kind="ExternalInput")
    wk = nc.dram_tensor("wk", [128, KC, INNER], F16, kind="ExternalInput")
    wv = nc.dram_tensor("wv", [128, KC, INNER], F16, kind="ExternalInput")
    wo = nc.dram_tensor("wo", [128, CI, DQ], F16, kind="ExternalInput")
    ones77 = nc.dram_tensor("ones77", [NK, 64], F16, kind="ExternalInput")
    out_l = nc.dram_tensor(
        "out_l", [BLOC, NTILES, 128, 4, DQ], F16, kind="ExternalOutput")

    with tile.TileContext(nc) as tc:
        with (
            tc.tile_pool(name="consts", bufs=1) as consts,
            tc.tile_pool(name="kv", bufs=1) as kv,
            tc.tile_pool(name="xp", bufs=3) as xp,
            tc.tile_pool(name="ep", bufs=12) as ep,
            tc.tile_pool(name="rp", bufs=8) as rp,
            tc.tile_pool(name="op", bufs=2) as op,
            tc.tile_pool(name="fp", bufs=3) as fp,
            tc.tile_pool(name="ps", bufs=8, space="PSUM") as ps,
        ):
            # ---- loads, ordered by first use; wq/xT0 split by k-chunk so
            # the first Q matmul starts as soon as chunk 0 lands.  The sync
            # queue dispatches DMAs ~3us later than scalar/vector/gpsimd at
            # startup, so the critical first chunks go on scalar+vector.
            # each dma_start costs ~1.7us of trigger latency on its queue, so
            # keep the startup loads UNSPLIT and spread by first-use across
            # the scalar/gpsimd queues (sync dispatches ~3us later).
            ones77_sb = consts.tile([NK, 64], F16)
            nc.gpsimd.dma_start(ones77_sb[:], ones77[:])
            wq_sb = consts.tile([128, KQ, INNER], F16)
            nc.scalar.dma_start(wq_sb[:], wq.ap())
            xT0_sb = xp.tile([128, KQ, TQ], F16, tag="xT")
            nc.sync.dma_start(xT0_sb[:], xT_l[0, 0])
            ctxT_sb = kv.tile([128, KC, BLOC * NK], F16)
            nc.scalar.dma_start(ctxT_sb[:], ctxT_l.ap())
            wk_sb = consts.tile([128, KC, INNER], F16)
            nc.gpsimd.dma_start(wk_sb[:], wk.ap())
            wv_sb = consts.tile([128, KC, INNER], F16)
            nc.gpsimd.dma_start(wv_sb[:], wv.ap())
            xT1_sb = xp.tile([128, KQ, TQ], F16, tag="xT")
            nc.sync.dma_start(xT1_sb[:], xT_l[0, 1])
            wo_sb = consts.tile([128, CI, DQ], F16)
            nc.sync.dma_start(wo_sb[:], wo.ap())

            # warm-up matmuls on ones77 ramp the PE clock a bit while the
            # big loads stream in
            warm_ps = ps.tile([64, 64], F32, tag="ps")
            for w in range(28):
                nc.tensor.matmul(
                    warm_ps[:], ones77_sb[:, 0:64], ones77_sb[:, 0:64],
                    start=(w == 0), stop=(w == 27))

            # ---- Q-proj of tile 0 first: only needs wq + xT0.  k-outer so
            # the first matmuls start when just the k=0 chunks have landed.
            qt0_sb = xp.tile([128, CI, TQ], F16, tag="qt")
            q0_pss = [ps.tile([128, TQ], F32, tag="ps", name=f"q0_ps{c}")
                      for c in range(CI)]
            for k in range(KQ):
                for c in range(CI):
                    nc.tensor.matmul(
                        q0_pss[c][:], wq_sb[:, k, c * 128:(c + 1) * 128],
                        xT0_sb[:, k, :], start=(k == 0), stop=(k == KQ - 1))
            for c in range(CI):
                nc.scalar.activation(
                    qt0_sb[:, c, :], q0_pss[c][:],
                    mybir.ActivationFunctionType.Copy)

            # ---- context projections (K^T both batches, V per batch).
            # k-outer so the first matmuls need only wk chunk 0.
            kt_sb = kv.tile([128, CI, BLOC * NK], F16)
            kt_pss = [ps.tile([128, BLOC * NK], F32, tag="ps",
                              name=f"kt_ps{c}") for c in range(CI)]
            for k in range(KC):
                for c in range(CI):
                    nc.tensor.matmul(
                        kt_pss[c][:], wk_sb[:, k, c * 128:(c + 1) * 128],
                        ctxT_sb[:, k, :], start=(k == 0), stop=(k == KC - 1))
            for c in range(CI):
                nc.scalar.activation(
                    kt_sb[:, c, :], kt_pss[c][:],
                    mybir.ActivationFunctionType.Copy)

            v_sb = kv.tile([NK, BLOC, INNER], F16)
            for b in range(BLOC):
                v_ps = ps.tile([NK, INNER], F32, tag="ps")
                for k in range(KC):
                    nc.tensor.matmul(
                        v_ps[:], ctxT_sb[:, k, b * NK:(b + 1) * NK],
                        wv_sb[:, k, :], start=(k == 0), stop=(k == KC - 1))
                nc.scalar.activation(
                    v_sb[:, b, :], v_ps[:],
                    mybir.ActivationFunctionType.Copy)

            # ---- main loop over (batch, nq tile), software-pipelined:
            # per iteration: S/exp(t), O/den/norm(t), Q-proj(t+1), out(t).
            # Q(t+1) between attention and out keeps the psum ring reusing
            # slots that were evicted mid-tile, and gives the PE work while
            # pair-3's normalization completes.
            NT = BLOC * NTILES

            def q_chunk(ti_next, qt_tag_sb, xT_sb, c):
                q_ps = ps.tile([128, TQ], F32, tag="ps",
                               name=f"q_ps_{ti_next}_{c}")
                for k in range(KQ):
                    nc.tensor.matmul(
                        q_ps[:], wq_sb[:, k, c * 128:(c + 1) * 128],
                        xT_sb[:, k, :], start=(k == 0), stop=(k == KQ - 1))
                nc.scalar.activation(
                    qt_tag_sb[:, c, :], q_ps[:],
                    mybir.ActivationFunctionType.Copy)

            qt_cur = qt0_sb
            for ti in range(NT):
                b, t = divmod(ti, NTILES)
                qt_sb = qt_cur

                # prefetch x^T of tile t+1 early
                if ti + 1 < NT:
                    bn, tn = divmod(ti + 1, NTILES)
                    if ti + 1 == 1:
                        xT_next = xT1_sb
                    else:
                        xT_next = xp.tile([128, KQ, TQ], F16, tag="xT",
                                          name=f"xT_{ti + 1}")
                        if (ti + 1) % 2 == 0:
                            nc.sync.dma_start(xT_next[:], xT_l[bn, tn])
                        else:
                            nc.gpsimd.dma_start(xT_next[:], xT_l[bn, tn])
                    qt_cur = xp.tile([128, CI, TQ], F16, tag="qt",
                                     name=f"qt_{ti + 1}")

                # attention: S^T, exp
                e_sbs = []
                for h in range(H):
                    c, r = h // 2, (h % 2) * 64
                    s_ps = ps.tile([NK, TQ], F32, tag="ps",
                                   name=f"s_ps_{ti}_{h}")
                    nc.tensor.matmul(
                        s_ps[:],
                        kt_sb[r:r + DH, c, b * NK:(b + 1) * NK],
                        qt_sb[r:r + DH, c, :])
                    e_sb = ep.tile([NK, TQ], F16, tag="expS",
                                   name=f"e_sb_{ti}_{h}")
                    nc.scalar.activation(
                        e_sb[:], s_ps[:], mybir.ActivationFunctionType.Exp)
                    e_sbs.append(e_sb)

                # O^T head-pairs packed [128, TQ], normalized by 1/den.
                # Q-proj chunks of tile t+1 are interleaved between pairs so
                # the PE never waits on the serial exp chain (pairs 2-3
                # would otherwise race exp5/exp7).
                ot_sb = op.tile([128, CI, TQ], F16, tag="ot",
                                name=f"ot_{ti}")
                for g in range(H // 2):
                    o2_ps = ps.tile([128, TQ], F32, tag="ps",
                                    name=f"o2_ps_{ti}_{g}")
                    d_ps = ps.tile([128, TQ], F32, tag="ps",
                                   name=f"d_ps_{ti}_{g}")
                    for half in range(2):
                        h = 2 * g + half
                        nc.tensor.matmul(
                            o2_ps[half * 64:(half + 1) * 64, :],
                            v_sb[:, b, h * DH:(h + 1) * DH],
                            e_sbs[h][:])
                        nc.tensor.matmul(
                            d_ps[half * 64:(half + 1) * 64, :],
                            ones77_sb[:], e_sbs[h][:],
                            tile_position=(0, half * 64))
                    rdbc = rp.tile([128, TQ], F32, tag="rdbc",
                                   name=f"rdbc_{ti}_{g}")
                    nc.vector.reciprocal_approx_fast(rdbc[:], d_ps[:])
                    nc.vector.tensor_mul(ot_sb[:, g, :], o2_ps[:], rdbc[:])
                    if ti + 1 < NT:
                        # one Q chunk of tile t+1 after every pair: keeps the
                        # PE off both the exp-chain race and the psum-ring
                        # WAR (pair allocs land ~1us after their slot frees)
                        q_chunk(ti + 1, qt_cur, xT_next, g)

                # out = (O^T).T @ Wo   (bias added on host)
                # c-outer / j-inner so the c=3 wave starts well after
                # pair-3's normalization.
                f_sb = fp.tile([128, 4, DQ], F16, tag="fin",
                               name=f"f_sb_{ti}")
                f_pss = [ps.tile([128, DQ], F32, tag="ps",
                                 name=f"f_ps_{ti}_{j}")
                         for j in range(4)]
                for c in range(CI):
                    for j in range(4):
                        nc.tensor.matmul(
                            f_pss[j][:], ot_sb[:, c, j * 128:(j + 1) * 128],
                            wo_sb[:, c, :],
                            start=(c == 0), stop=(c == CI - 1))
                # final evictions on DVE (keeping ACT clear for the next
                # tile's exp chain); last tile splits ACT/DVE and streams
                # each chunk out as soon as it is evicted
                last = ti == NT - 1
                for j in range(4):
                    if last and j % 2 == 0:
                        nc.scalar.activation(
                            f_sb[:, j, :], f_pss[j][:],
                            mybir.ActivationFunctionType.Copy)
                    else:
                        nc.vector.tensor_copy(f_sb[:, j, :], f_pss[j][:])
                    if last:
                        # spread trigger latency across queues
                        eng = [nc.sync, nc.scalar, nc.gpsimd, nc.sync][j]
                        eng.dma_start(out_l[b, t, :, j], f_sb[:, j, :])
                if not last:
                    st_eng = [nc.sync, nc.gpsimd][ti % 2]
                    st_eng.dma_start(out_l[b, t], f_sb[:])

    nc.compile()
    return nc


_NC_CACHE = {}


def _get_nc():
    if "nc" not in _NC_CACHE:
        _NC_CACHE["nc"] = _build_nc()
    return _NC_CACHE["nc"]


def _make_in_maps(x, context, Wq, Wk, Wv, Wo):
    f = np.float32

    def wtile(w, kchunks):
        # [K, N] -> [128, kchunks, N] with partition p holding rows
        # {p, 128+p, ...}? No: row k*128+p -> [p, k, n]
        w = np.ascontiguousarray(w, dtype=f).astype(np.float16)
        return np.ascontiguousarray(
            w.reshape(kchunks, 128, -1).transpose(1, 0, 2))

    wq_t = np.ascontiguousarray(Wq, dtype=f) * np.float32(SCALE)
    shared = {
        "wq": wtile(wq_t, KQ),
        "wk": wtile(np.asarray(Wk, dtype=f), KC),
        "wv": wtile(np.asarray(Wv, dtype=f), KC),
        "wo": wtile(np.asarray(Wo, dtype=f), CI),
        "ones77": np.ones((NK, 64), dtype=np.float16),
    }
    in_maps = []
    for i in range(NCORES):
        m = dict(shared)
        xc = np.asarray(x[BLOC * i:BLOC * (i + 1)], dtype=f)
        # x [b, nq, dq] -> xT [b, t, p, c, n] = x[b, t*TQ+n, c*128+p]
        xt = xc.reshape(BLOC, NTILES, TQ, KQ, 128).transpose(0, 1, 4, 3, 2)
        m["xT_l"] = np.ascontiguousarray(xt).astype(np.float16)
        cc = np.asarray(context[BLOC * i:BLOC * (i + 1)], dtype=f)
        # ctx [b, nk, dc] -> ctxT [p, k, b*NK+n] = ctx[b, n, k*128+p]
        ct = cc.reshape(BLOC, NK, KC, 128).transpose(3, 2, 0, 1).reshape(
            128, KC, BLOC * NK)
        m["ctxT_l"] = np.ascontiguousarray(ct).astype(np.float16)
        in_maps.append(m)
    return in_maps


def run(x, context, Wq, Wk, Wv, Wo, bo, trace=False, **trace_kwargs):
    nc = _get_nc()
    in_maps = _make_in_maps(x, context, Wq, Wk, Wv, Wo)
    res = run_bass_kernel_spmd(
        nc, in_maps, list(range(NCORES)), trace=trace, **trace_kwargs)
    parts = []
    for i in range(NCORES):
        o = np.asarray(res.results[i]["out_l"])  # [BLOC, NTILES, 128, 4, DQ]
        o = o.transpose(0, 1, 3, 2, 4).reshape(BLOC, NQ, DQ)
        parts.append(o)
    out = np.concatenate(parts, axis=0).astype(np.float32)
    out += np.asarray(bo, dtype=np.float32)[None, None, :]
    return out, res


def kernel(x, context, Wq, Wk, Wv, Wo, bo):
    out, _ = run(x, context, Wq, Wk, Wv, Wo, bo, trace=False)
    return out


# revision 24
# speedup vs baseline: 1.0218x; 1.0059x over previous
"""CrossAttention kernel for 8 Trainium2 NeuronCores.

Data-parallel over batch: B=16 batches -> 2 per core. Each core computes the
full cross-attention for its 2 batches; outputs are concatenated on host.

Per-core dataflow (all matmuls fp16 in / fp32 psum, full-rate on the PE):
  x^T    host-pretiled [b, t, 128, 4, TQ]
  Q^T    = Wq^T @ x^T            (Wq pre-scaled by 1/8 on host)
  ctx^T  host-pretransposed -> K^T = Wk^T @ ctx^T,  V = ctx @ Wv
  S^T    = (K_h^T).T @ Q_h^T     [77, nq]  per head
  expS   = exp(S^T)              (no max-subtract; |S| <~ 6 so fp32 exp safe)
  den    = ones^T @ expS         [64, nq] replicated rows, pair-packed psum
  O^T    = V_h.T @ expS          [64, nq], head pairs packed into [128, nq]
  norm   : O^T * recip(den)      (DVE)
  out    = (O^T).T @ Wo          [nq, 512] fp16 tile-major; host adds bias
           and casts to fp32.

All DRAM tensors are host-side pre-tiled so each DMA is contiguous >=2KB per
partition (few descriptors). PE program starts with Q-proj of tile 0 (needs
only wq+xT0) to cover the wk/wv/wo load latency.

Schedule notes (from perfetto traces):
- PE cost ~= (ldweights cols + rhs rows)/2.4GHz per matmul; LDWEIGHTS is
  always emitted 1:1 with matmuls (walrus ldw-opt disabled) but hides in the
  load-while-execute shadow when the queue is saturated.
- Main loop is software-pipelined: S/exp(t), then [O/den pair g + Q-proj
  chunk g of tile t+1] interleaved, then out-proj(t) c-outer/j-inner.  The
  interleave keeps the PE off both the serial ACT exp chain and psum-ring
  WAR stalls (24 psum allocs/tile cycle through 8 banks).
- Evictions: qt+exp on ACT, recip/mul/final on DVE, DMA triggers on
  sync/scalar/gpsimd queues only (a dma_start costs ~0.7-1.7us of trigger
  latency on its issuing queue).
"""

import os
import sys

for _p in ("/opt/trn_rl_repo",):
    if _p not in sys.path:
        sys.path.insert(0, _p)

import numpy as np

import concourse.bass as bass
import concourse.bacc as bacc
import concourse.mybir as mybir
import concourse.tile as tile
from concourse.bass_utils import run_bass_kernel_spmd

# Problem constants (hardcoded per contract)
B, NQ, NK = 16, 4096, 77
DQ, DC = 512, 768
H, DH = 8, 64
INNER = H * DH  # 512
SCALE = DH ** -0.5  # 1/8
NCORES = 8
BLOC = B // NCORES  # 2 batches per core

F32 = mybir.dt.float32
F16 = mybir.dt.float16

TQ = 512          # nq tile (free dim of most matmuls)
NTILES = NQ // TQ  # 8 per batch
KQ = DQ // 128    # 4 contraction chunks for Wq
KC = DC // 128    # 6 contraction chunks for Wk/Wv
CI = INNER // 128  # 4 inner chunks


def _build_nc():
    nc = bacc.Bacc("TRN2", target_bir_lowering=False, debug=False)

    # host-pretiled inputs: contiguous per-partition rows
    xT_l = nc.dram_tensor(
        "xT_l", [BLOC, NTILES, 128, KQ, TQ], F16, kind="ExternalInput")
    ctxT_l = nc.dram_tensor(
        "ctxT_l", [128, KC, BLOC * NK], F16, kind="ExternalInput")
    wq = nc.dram_tensor("wq", [128, KQ, INNER], F16, kind="ExternalInput")
    wk = nc.dram_tensor("wk", [128, KC, INNER], F16, kind="ExternalInput")
    wv = nc.dram_tensor("wv", [128, KC, INNER], F16, kind="ExternalInput")
    wo = nc.dram_tensor("wo", [128, CI, DQ], F16, kind="ExternalInput")
    ones77 = nc.dram_tensor("ones77", [NK, 64], F16, kind="ExternalInput")
    out_l = nc.dram_tensor(
        "out_l", [BLOC, NTILES, 128, 4, DQ], F16, kind="ExternalOutput")

    with tile.TileContext(nc) as tc:
        with (
            tc.tile_pool(name="consts", bufs=1) as consts,
            tc.tile_pool(name="kv", bufs=1) as kv,
            tc.tile_pool(name="xp", bufs=3) as xp,
            tc.tile_pool(name="ep", bufs=12) as ep,
            tc.tile_pool(name="rp", bufs=8) as rp,
            tc.tile_pool(name="op", bufs=2) as op,
            tc.tile_pool(name="fp", bufs=3) as fp,
            tc.tile_pool(name="ps", bufs=8, space="PSUM") as ps,
        ):
            # ---- loads, ordered by first use; wq/xT0 split by k-chunk so
            # the first Q matmul starts as soon as chunk 0 lands.  The sync
            # queue dispatches DMAs ~3us later than scalar/vector/gpsimd at
            # startup, so the critical first chunks go on scalar+vector.
            # each dma_start costs ~1.7us of trigger latency on its queue, so
            # keep the startup loads UNSPLIT and spread by first-use across
            # the scalar/gpsimd queues (sync dispatches ~3us later).
            ones77_sb = consts.tile([NK, 64], F16)
            nc.gpsimd.dma_start(ones77_sb[:], ones77[:])
            wq_sb = consts.tile([128, KQ, INNER], F16)
            nc.scalar.dma_start(wq_sb[:], wq.ap())
            xT0_sb = xp.tile([128, KQ, TQ], F16, tag="xT")
            nc.sync.dma_start(xT0_sb[:], xT_l[0, 0])
            ctxT_sb = kv.tile([128, KC, BLOC * NK], F16)
            nc.scalar.dma_start(ctxT_sb[:], ctxT_l.ap())
            wk_sb = consts.tile([128, KC, INNER], F16)
            nc.gpsimd.dma_start(wk_sb[:], wk.ap())
            wv_sb = consts.tile([128, KC, INNER], F16)
            nc.gpsimd.dma_start(wv_sb[:], wv.ap())
            xT1_sb = xp.tile([128, KQ, TQ], F16, tag="xT")
            nc.sync.dma_start(xT1_sb[:], xT_l[0, 1])
            wo_sb = consts.tile([128, CI, DQ], F16)
            nc.sync.dma_start(wo_sb[:], wo.ap())

            # warm-up matmuls on ones77 ramp the PE clock a bit while the
            # big loads stream in
            warm_ps = ps.tile([64, 64], F32, tag="ps")
            for w in range(28):
                nc.tensor.matmul(
                    warm_ps[:], ones77_sb[:, 0:64], ones77_sb[:, 0:64],
                    start=(w == 0), stop=(w == 27))

            # ---- Q-proj of tile 0 first: only needs wq + xT0.  k-outer so
            # the first matmuls start when just the k=0 chunks have landed.
            qt0_sb = xp.tile([128, CI, TQ], F16, tag="qt")
            q0_pss = [ps.tile([128, TQ], F32, tag="ps", name=f"q0_ps{c}")
                      for c in range(CI)]
            for k in range(KQ):
                for c in range(CI):
                    nc.tensor.matmul(
                        q0_pss[c][:], wq_sb[:, k, c * 128:(c + 1) * 128],
                        xT0_sb[:, k, :], start=(k == 0), stop=(k == KQ - 1))
            for c in range(CI):
                nc.scalar.activation(
                    qt0_sb[:, c, :], q0_pss[c][:],
                    mybir.ActivationFunctionType.Copy)

            # ---- context projections (K^T both batches, V per batch).
            # k-outer so the first matmuls need only wk chunk 0.
            kt_sb = kv.tile([128, CI, BLOC * NK], F16)
            kt_pss = [ps.tile([128, BLOC * NK], F32, tag="ps",
                              name=f"kt_ps{c}") for c in range(CI)]
            for k in range(KC):
                for c in range(CI):
                    nc.tensor.matmul(
                        kt_pss[c][:], wk_sb[:, k, c * 128:(c + 1) * 128],
                        ctxT_sb[:, k, :], start=(k == 0), stop=(k == KC - 1))
            for c in range(CI):
                nc.scalar.activation(
                    kt_sb[:, c, :], kt_pss[c][:],
                    mybir.ActivationFunctionType.Copy)

            v_sb = kv.tile([NK, BLOC, INNER], F16)
            for b in range(BLOC):
                v_ps = ps.tile([NK, INNER], F32, tag="ps")
                for k in range(KC):
                    nc.tensor.matmul(
                        v_ps[:], ctxT_sb[:, k, b * NK:(b + 1) * NK],
                        wv_sb[:, k, :], start=(k == 0), stop=(k == KC - 1))
                nc.scalar.activation(
                    v_sb[:, b, :], v_ps[:],
                    mybir.ActivationFunctionType.Copy)

            # ---- main loop over (batch, nq tile), software-pipelined:
            # per iteration: S/exp(t), O/den/norm(t), Q-proj(t+1), out(t).
            # Q(t+1) between attention and out keeps the psum ring reusing
            # slots that were evicted mid-tile, and gives the PE work while
            # pair-3's normalization completes.
            NT = BLOC * NTILES

            def q_chunk(ti_next, qt_tag_sb, xT_sb, c):
                q_ps = ps.tile([128, TQ], F32, tag="ps",
                               name=f"q_ps_{ti_next}_{c}")
                for k in range(KQ):
                    nc.tensor.matmul(
                        q_ps[:], wq_sb[:, k, c * 128:(c + 1) * 128],
                        xT_sb[:, k, :], start=(k == 0), stop=(k == KQ - 1))
                nc.scalar.activation(
                    qt_tag_sb[:, c, :], q_ps[:],
                    mybir.ActivationFunctionType.Copy)

            qt_cur = qt0_sb
            for ti in range(NT):
                b, t = divmod(ti, NTILES)
                qt_sb = qt_cur

                # prefetch x^T of tile t+1 early
                if ti + 1 < NT:
                    bn, tn = divmod(ti + 1, NTILES)
                    if ti + 1 == 1:
                        xT_next = xT1_sb
                    else:
                        xT_next = xp.tile([128, KQ, TQ], F16, tag="xT",
                                          name=f"xT_{ti + 1}")
                        if (ti + 1) % 2 == 0:
                            nc.sync.dma_start(xT_next[:], xT_l[bn, tn])
                        else:
                            nc.gpsimd.dma_start(xT_next[:], xT_l[bn, tn])
                    qt_cur = xp.tile([128, CI, TQ], F16, tag="qt",
                                     name=f"qt_{ti + 1}")

                # attention: S^T, exp
                e_sbs = []
                for h in range(H):
                    c, r = h // 2, (h % 2) * 64
                    s_ps = ps.tile([NK, TQ], F32, tag="ps",
                                   name=f"s_ps_{ti}_{h}")
                    nc.tensor.matmul(
                        s_ps[:],
                        kt_sb[r:r + DH, c, b * NK:(b + 1) * NK],
                        qt_sb[r:r + DH, c, :])
                    e_sb = ep.tile([NK, TQ], F16, tag="expS",
                                   name=f"e_sb_{ti}_{h}")
                    nc.scalar.activation(
                        e_sb[:], s_ps[:], mybir.ActivationFunctionType.Exp)
                    e_sbs.append(e_sb)

                # O^T head-pairs packed [128, TQ], normalized by 1/den.
                # Q-proj chunks of tile t+1 are interleaved between pairs so
                # the PE never waits on the serial exp chain (pairs 2-3
                # would otherwise race exp5/exp7).
                ot_sb = op.tile([128, CI, TQ], F16, tag="ot",
                                name=f"ot_{ti}")
                for g in range(H // 2):
                    o2_ps = ps.tile([128, TQ], F32, tag="ps",
                                    name=f"o2_ps_{ti}_{g}")
                    d_ps = ps.tile([128, TQ], F32, tag="ps",
                                   name=f"d_ps_{ti}_{g}")
                    for half in range(2):
                        h = 2 * g + half
                        nc.tensor.matmul(
                            o2_ps[half * 64:(half + 1) * 64, :],
                            v_sb[:, b, h * DH:(h + 1) * DH],
                            e_sbs[h][:])
                        nc.tensor.matmul(
                            d_ps[half * 64:(half + 1) * 64, :],
                            ones77_sb[:], e_sbs[h][:],
                            tile_position=(0, half * 64))
                    rdbc = rp.tile([128, TQ], F32, tag="rdbc",
                                   name=f"rdbc_{ti}_{g}")
                    nc.vector.reciprocal_approx_fast(rdbc[:], d_ps[:])
                    nc.vector.tensor_mul(ot_sb[:, g, :], o2_ps[:], rdbc[:])
                    if ti + 1 < NT:
                        # one Q chunk of tile t+1 after every pair: keeps the
                        # PE off both the exp-chain race and the psum-ring
                        # WAR (pair allocs land ~1us after their slot frees)
                        q_chunk(ti + 1, qt_cur, xT_next, g)

                # out = (O^T).T @ Wo   (bias added on host)
                # c-outer / j-inner so the c=3 wave starts well after
                # pair-3's normalization.
                f_sb = fp.tile([128, 4, DQ], F16, tag="fin",
                               name=f"f_sb_{ti}")
                f_pss = [ps.tile([128, DQ], F32, tag="ps",
                                 name=f"f_ps_{ti}_{j}")
                         for j in range(4)]
                for c in range(CI):
                    for j in range(4):
                        nc.tensor.matmul(
                            f_pss[j][:], ot_sb[:, c, j * 128:(j + 1) * 128],
                            wo_sb[:, c, :],
                            start=(c == 0), stop=(c == CI - 1))
                # final evictions on DVE (keeping ACT clear for the next
                # tile's exp chain); last tile splits ACT/DVE and streams
                # each chunk out as soon as it is evicted
                last = ti == NT - 1
                for j in range(4):
                    if last and j % 2 == 0:
                        nc.scalar.activation(
                            f_sb[:, j, :], f_pss[j][:],
                            mybir.ActivationFunctionType.Copy)
                    else:
                        nc.vector.tensor_copy(f_sb[:, j, :], f_pss[j][:])
                    if last:
                        # spread trigger latency across queues
                        eng = [nc.sync, nc.scalar, nc.gpsimd, nc.sync][j]
                        eng.dma_start(out_l[b, t, :, j], f_sb[:, j, :])
                if not last:
                    st_eng = [nc.sync, nc.gpsimd][ti % 2]
                    st_eng.dma_start(out_l[b, t], f_sb[:])

    nc.compile()
    return nc


_NC_CACHE = {}


def _get_nc():
    if "nc" not in _NC_CACHE:
        _NC_CACHE["nc"] = _build_nc()
    return _NC_CACHE["nc"]


def _make_in_maps(x, context, Wq, Wk, Wv, Wo):
    f = np.float32

    def wtile(w, kchunks):
        # [K, N] -> [128, kchunks, N] with partition p holding rows
        # {p, 128+p, ...}? No: row k*128+p -> [p, k, n]
        w = np.ascontiguousarray(w, dtype=f).astype(np.float16)
        return np.ascontiguousarray(
            w.reshape(kchunks, 128, -1).transpose(1, 0, 2))

    wq_t = np.ascontiguousarray(Wq, dtype=f) * np.float32(SCALE)
    shared = {
        "wq": wtile(wq_t, KQ),
        "wk": wtile(np.asarray(Wk, dtype=f), KC),
        "wv": wtile(np.asarray(Wv, dtype=f), KC),
        "wo": wtile(np.asarray(Wo, dtype=f), CI),
        "ones77": np.ones((NK, 64), dtype=np.float16),
    }
    in_maps = []
    for i in range(NCORES):
        m = dict(shared)
        xc = np.asarray(x[BLOC * i:BLOC * (i + 1)], dtype=f)
        # x [b, nq, dq] -> xT [b, t, p, c, n] = x[b, t*TQ+n, c*128+p]
        xt = xc.reshape(BLOC, NTILES, TQ, KQ, 128).transpose(0, 1, 4, 3, 2)
        m["xT_l"] = np.ascontiguousarray(xt).astype(np.float16)
        cc = np.asarray(context[BLOC * i:BLOC * (i + 1)], dtype=f)
        # ctx [b, nk, dc] -> ctxT [p, k, b*NK+n] = ctx[b, n, k*128+p]
        ct = cc.reshape(BLOC, NK, KC, 128).transpose(3, 2, 0, 1).reshape(
            128, KC, BLOC * NK)
        m["ctxT_l"] = np.ascontiguousarray(ct).astype(np.float16)
        in_maps.append(m)
    return in_maps


def run(x, context, Wq, Wk, Wv, Wo, bo, trace=False, **trace_kwargs):
    nc = _get_nc()
    in_maps = _make_in_maps(x, context, Wq, Wk, Wv, Wo)
    res = run_bass_kernel_spmd(
        nc, in_maps, list(range(NCORES)), trace=trace, **trace_kwargs)
    parts = []
    for i in range(NCORES):
        o = np.asarray(res.results[i]["out_l"])  # [BLOC, NTILES, 128, 4, DQ]
        o = o.transpose(0, 1, 3, 2, 4).reshape(BLOC, NQ, DQ)
        parts.append(o)
    out = np.concatenate(parts, axis=0).astype(np.float32)
    out += np.asarray(bo, dtype=np.float32)[None, None, :]
    return out, res


def kernel(x, context, Wq, Wk, Wv, Wo, bo):
    out, _ = run(x, context, Wq, Wk, Wv, Wo, bo, trace=False)
    return out
